# revision 6
# baseline (speedup 1.0000x reference)
"""Trainium2 Bass kernel for nn_CLCRec_Graph loss.

Data-parallel across 8 NeuronCores: core c owns batch rows [128c, 128c+128).
Each core gathers its embedding/content rows via indirect DMA, runs the
2-layer encoder on just its gathered rows (feature-major via PE transposes),
computes both contrastive losses + reg norms, and writes [128, 4] partial
sums. Host combines partials into the two output scalars.
"""
import sys
import os
import functools

for _p in ("/opt/trn_rl_repo", "/root/.axon_site/_ro/trn_rl_repo"):
    if os.path.isdir(_p) and _p not in sys.path:
        sys.path.insert(0, _p)

import numpy as np

from concourse import bass, bacc, mybir
import concourse.tile as tile
from concourse.masks import make_identity

NUM_USER = 100000
NUM_ITEM = 50000
DIM_E = 128
DIM_FEAT = 256
BATCH = 1024
J = 128            # 1 + NUM_NEG slots per batch row
TEMP = 2.0
LR_LAMBDA = 0.5
N_CORES = 8
BC = BATCH // N_CORES   # batch rows per core = 128
JC = 8                  # slots per chunk
NCHUNK = J // JC        # 16 chunks
FLAT = BATCH * J

F32 = mybir.dt.float32
I32 = mybir.dt.int32


def build_program():
    nc = bacc.Bacc("TRN2", target_bir_lowering=False)

    uidx = nc.dram_tensor("uidx", [BC, J], I32, kind="ExternalInput")
    iidx = nc.dram_tensor("iidx", [BC, J], I32, kind="ExternalInput")
    vidx = nc.dram_tensor("vidx", [BC, J], I32, kind="ExternalInput")
    pidx = nc.dram_tensor("pidx", [BC, 1], I32, kind="ExternalInput")
    maskd = nc.dram_tensor("maskd", [BC, J], F32, kind="ExternalInput")
    id_emb = nc.dram_tensor("id_emb", [NUM_USER + NUM_ITEM, DIM_E], F32, kind="ExternalInput")
    v_feat = nc.dram_tensor("v_feat", [NUM_ITEM, DIM_FEAT], F32, kind="ExternalInput")
    w1 = nc.dram_tensor("w1", [DIM_FEAT, DIM_FEAT], F32, kind="ExternalInput")
    w2 = nc.dram_tensor("w2", [DIM_FEAT, DIM_E], F32, kind="ExternalInput")
    b1 = nc.dram_tensor("b1", [DIM_FEAT], F32, kind="ExternalInput")
    b2 = nc.dram_tensor("b2", [DIM_E], F32, kind="ExternalInput")
    out = nc.dram_tensor("partials", [BC, 4], F32, kind="ExternalOutput")

    with tile.TileContext(nc) as tc:
        _body(nc, tc, uidx, iidx, vidx, pidx, maskd, id_emb, v_feat, w1, w2, b1, b2, out)
    nc.compile()
    return nc


def _body(nc, tc, uidx, iidx, vidx, pidx, maskd, id_emb, v_feat, w1, w2, b1, b2, out):
    from contextlib import ExitStack

    with ExitStack() as ctx:
        const = ctx.enter_context(tc.tile_pool(name="const", bufs=1))
        acc = ctx.enter_context(tc.tile_pool(name="acc", bufs=1))
        work = ctx.enter_context(tc.tile_pool(name="work", bufs=2))
        psum = ctx.enter_context(tc.tile_pool(name="psum", bufs=4, space="PSUM"))

        # ---- constants / prologue ----
        ident = const.tile([128, 128], F32)
        make_identity(nc, ident[:])

        w1sb = const.tile([128, 2, DIM_FEAT], F32)   # [p, kblock, out_feat]
        nc.sync.dma_start(out=w1sb[:], in_=w1.rearrange("(k p) m -> p k m", p=128))
        w2sb = const.tile([128, 2, DIM_E], F32)
        nc.sync.dma_start(out=w2sb[:], in_=w2.rearrange("(k p) m -> p k m", p=128))
        b1sb = const.tile([128, 2], F32)
        nc.sync.dma_start(out=b1sb[:], in_=b1.rearrange("(m p) -> p m", p=128))
        b2sb = const.tile([128, 1], F32)
        nc.sync.dma_start(out=b2sb[:], in_=b2.rearrange("(m p) -> p m", p=128))

        uidx_t = const.tile([BC, J], I32)
        nc.sync.dma_start(out=uidx_t[:], in_=uidx[:, :])
        iidx_t = const.tile([BC, J], I32)
        nc.sync.dma_start(out=iidx_t[:], in_=iidx[:, :])
        vidx_t = const.tile([BC, J], I32)
        nc.sync.dma_start(out=vidx_t[:], in_=vidx[:, :])
        pidx_t = const.tile([BC, 1], I32)
        nc.sync.dma_start(out=pidx_t[:], in_=pidx[:, :])
        mask_t = const.tile([BC, J], F32)
        nc.sync.dma_start(out=mask_t[:], in_=maskd[:, :])

        # anchor rows: pos_emb -> l2-normalized, row-major [b, E]
        pos_t = const.tile([BC, DIM_E], F32)
        nc.gpsimd.indirect_dma_start(
            out=pos_t[:], out_offset=None, in_=id_emb[:],
            in_offset=bass.IndirectOffsetOnAxis(ap=pidx_t[:, :1], axis=0),
        )
        scr_h = work.tile([BC, DIM_E], F32, tag="scr_h")
        nh2 = work.tile([BC, 1], F32, tag="nh")
        nc.vector.tensor_tensor(out=scr_h[:], in0=pos_t[:], in1=pos_t[:], op=mybir.AluOpType.mult)
        nc.vector.reduce_sum(out=nh2[:], in_=scr_h[:], axis=mybir.AxisListType.X)
        nh = work.tile([BC, 1], F32, tag="nh")
        nc.scalar.sqrt(nh[:], nh2[:])
        invh = const.tile([BC, 1], F32)
        nc.vector.reciprocal(invh[:], nh[:])
        hhat = const.tile([BC, DIM_E], F32)
        nc.vector.tensor_scalar_mul(hhat[:], pos_t[:], invh[:, :1])

        # accumulators [b, j]
        d1 = acc.tile([BC, J], F32)
        d2e = acc.tile([BC, J], F32)
        d2f = acc.tile([BC, J], F32)
        nf2 = acc.tile([BC, J], F32)
        nu2 = acc.tile([BC, J], F32)
        ne2 = acc.tile([BC, J], F32)

        for c in range(NCHUNK):
            js = slice(c * JC, (c + 1) * JC)

            # ---- gathers (row-major [b, (jl, feat)]) ----
            x_t = work.tile([BC, JC * DIM_FEAT], F32, tag="x")
            nc.gpsimd.indirect_dma_start(
                out=x_t[:], out_offset=None, in_=v_feat[:],
                in_offset=bass.IndirectOffsetOnAxis(ap=vidx_t[:, js], axis=0),
            )
            u_t = work.tile([BC, JC * DIM_E], F32, tag="u")
            nc.gpsimd.indirect_dma_start(
                out=u_t[:], out_offset=None, in_=id_emb[:],
                in_offset=bass.IndirectOffsetOnAxis(ap=uidx_t[:, js], axis=0),
            )
            e_t = work.tile([BC, JC * DIM_E], F32, tag="e")
            nc.gpsimd.indirect_dma_start(
                out=e_t[:], out_offset=None, in_=id_emb[:],
                in_offset=bass.IndirectOffsetOnAxis(ap=iidx_t[:, js], axis=0),
            )

            # ---- v_feat row l2 norms ----
            scr_x = work.tile([BC, JC * DIM_FEAT], F32, tag="scrx")
            nc.vector.tensor_tensor(out=scr_x[:], in0=x_t[:], in1=x_t[:], op=mybir.AluOpType.mult)
            nx2 = work.tile([BC, JC], F32, tag="nx")
            nc.vector.reduce_sum(
                out=nx2[:], in_=scr_x[:].rearrange("p (j f) -> p j f", f=DIM_FEAT),
                axis=mybir.AxisListType.X,
            )
            nx = work.tile([BC, JC], F32, tag="nx")
            nc.scalar.sqrt(nx[:], nx2[:])
            invn = work.tile([BC, JC], F32, tag="invn")
            nc.vector.reciprocal(invn[:], nx[:])

            # diag_j = identity * invn_j  (one fused op over the chunk)
            diag_t = work.tile([BC, JC * 128], F32, tag="diag")
            nc.vector.tensor_tensor(
                out=diag_t[:].rearrange("p (j q) -> p j q", q=128),
                in0=ident[:].rearrange("p (o q) -> p o q", o=1).to_broadcast([128, JC, 128]),
                in1=invn[:].rearrange("p (j o) -> p j o", o=1).to_broadcast([128, JC, 128]),
                op=mybir.AluOpType.mult,
            )

            # ---- transpose+normalize: Xt[fb][:, jl*128+b] = x_hat^T ----
            xt_ps = [psum.tile([128, JC * 128], F32, space="PSUM", tag="ps", name=f"xtps{fb}") for fb in range(2)]
            for jl in range(JC):
                for fb in range(2):
                    nc.tensor.matmul(
                        out=xt_ps[fb][:, jl * 128:(jl + 1) * 128],
                        lhsT=x_t[:, jl * DIM_FEAT + fb * 128: jl * DIM_FEAT + (fb + 1) * 128],
                        rhs=diag_t[:, jl * 128:(jl + 1) * 128],
                        start=True, stop=True,
                    )
            xt = [work.tile([128, JC * 128], F32, tag=f"xt{fb}", name=f"xt{fb}") for fb in range(2)]
            for fb in range(2):
                nc.vector.tensor_copy(xt[fb][:], xt_ps[fb][:])

            # ---- MLP layer 1: h1[m] = lrelu(W1[:,m]^T @ xhat^T + b1[m]) ----
            NT = JC * 128 // 512  # 512-wide matmul tiles
            h1_ps = [psum.tile([128, JC * 128], F32, space="PSUM", tag="ps", name=f"h1ps{m}") for m in range(2)]
            for m in range(2):
                for nt in range(NT):
                    ns = slice(nt * 512, (nt + 1) * 512)
                    for kb in range(2):
                        nc.tensor.matmul(
                            out=h1_ps[m][:, ns],
                            lhsT=w1sb[:, kb, m * 128:(m + 1) * 128],
                            rhs=xt[kb][:, ns],
                            start=(kb == 0), stop=(kb == 1),
                        )
            h1 = [work.tile([128, JC * 128], F32, tag=f"h1{m}", name=f"h1{m}") for m in range(2)]
            z1 = [work.tile([128, JC * 128], F32, tag=f"z1{m}", name=f"z1{m}") for m in range(2)]
            for m in range(2):
                nc.scalar.activation(
                    out=z1[m][:], in_=h1_ps[m][:],
                    func=mybir.ActivationFunctionType.Identity,
                    bias=b1sb[:, m:m + 1], scale=1.0,
                )
                nc.vector.scalar_tensor_tensor(
                    out=h1[m][:], in0=z1[m][:], scalar=0.01, in1=z1[m][:],
                    op0=mybir.AluOpType.mult, op1=mybir.AluOpType.max,
                )

            # ---- MLP layer 2: Ft = W2^T @ h1 + b2 (feature-major [E, (jl, b)]) ----
            f_ps = psum.tile([128, JC * 128], F32, space="PSUM", tag="ps")
            for nt in range(NT):
                ns = slice(nt * 512, (nt + 1) * 512)
                for kb in range(2):
                    nc.tensor.matmul(
                        out=f_ps[:, ns],
                        lhsT=w2sb[:, kb, :],
                        rhs=h1[kb][:, ns],
                        start=(kb == 0), stop=(kb == 1),
                    )
            ft = work.tile([128, JC * 128], F32, tag="ft")
            nc.scalar.activation(
                out=ft[:], in_=f_ps[:],
                func=mybir.ActivationFunctionType.Identity,
                bias=b2sb[:, :1], scale=1.0,
            )

            # ---- transpose back to rows: F_rows [b, (jl, E)] ----
            fr_ps = psum.tile([128, JC * 128], F32, space="PSUM", tag="ps")
            for jl in range(JC):
                nc.tensor.transpose(
                    out=fr_ps[:, jl * 128:(jl + 1) * 128],
                    in_=ft[:, jl * 128:(jl + 1) * 128],
                    identity=ident[:],
                )
            f_rows = work.tile([BC, JC * DIM_E], F32, tag="frows")
            nc.vector.tensor_copy(f_rows[:], fr_ps[:])

            # ---- c1 dots: d1[b, j] = <hhat_b, F_{b,j}> ----
            scr = work.tile([BC, JC * DIM_E], F32, tag="scr")
            nc.vector.tensor_tensor(
                out=scr[:].rearrange("p (j f) -> p j f", f=DIM_E),
                in0=f_rows[:].rearrange("p (j f) -> p j f", f=DIM_E),
                in1=hhat[:].rearrange("p (o f) -> p o f", o=1).to_broadcast([BC, JC, DIM_E]),
                op=mybir.AluOpType.mult,
            )
            nc.vector.reduce_sum(
                out=d1[:, js], in_=scr[:].rearrange("p (j f) -> p j f", f=DIM_E),
                axis=mybir.AxisListType.X,
            )
            # ---- F row norms^2 ----
            scr2 = work.tile([BC, JC * DIM_E], F32, tag="scr2")
            nc.vector.tensor_tensor(out=scr2[:], in0=f_rows[:], in1=f_rows[:], op=mybir.AluOpType.mult)
            nc.vector.reduce_sum(
                out=nf2[:, js], in_=scr2[:].rearrange("p (j f) -> p j f", f=DIM_E),
                axis=mybir.AxisListType.X,
            )
            # ---- reg norms^2 for u and e ----
            nc.vector.tensor_tensor(out=scr[:], in0=u_t[:], in1=u_t[:], op=mybir.AluOpType.mult)
            nc.vector.reduce_sum(
                out=nu2[:, js], in_=scr[:].rearrange("p (j f) -> p j f", f=DIM_E),
                axis=mybir.AxisListType.X,
            )
            nc.vector.tensor_tensor(out=scr2[:], in0=e_t[:], in1=e_t[:], op=mybir.AluOpType.mult)
            nc.vector.reduce_sum(
                out=ne2[:, js], in_=scr2[:].rearrange("p (j f) -> p j f", f=DIM_E),
                axis=mybir.AxisListType.X,
            )
            # ---- c2 dots: both variants, scalar select happens in epilogue ----
            nc.vector.tensor_tensor(out=scr[:], in0=u_t[:], in1=e_t[:], op=mybir.AluOpType.mult)
            nc.vector.reduce_sum(
                out=d2e[:, js], in_=scr[:].rearrange("p (j f) -> p j f", f=DIM_E),
                axis=mybir.AxisListType.X,
            )
            nc.vector.tensor_tensor(out=scr2[:], in0=u_t[:], in1=f_rows[:], op=mybir.AluOpType.mult)
            nc.vector.reduce_sum(
                out=d2f[:, js], in_=scr2[:].rearrange("p (j f) -> p j f", f=DIM_E),
                axis=mybir.AxisListType.X,
            )

        # ---- epilogue ----
        # d2 = d2e + mask * (d2f - d2e)
        d2 = acc.tile([BC, J], F32)
        nc.vector.tensor_sub(out=d2[:], in0=d2f[:], in1=d2e[:])
        nc.vector.tensor_mul(out=d2[:], in0=d2[:], in1=mask_t[:])
        nc.vector.tensor_add(out=d2[:], in0=d2[:], in1=d2e[:])

        nf = acc.tile([BC, J], F32)
        nc.scalar.sqrt(nf[:], nf2[:])
        invf = acc.tile([BC, J], F32)
        nc.vector.reciprocal(invf[:], nf[:])
        a1 = acc.tile([BC, J], F32)
        nc.vector.tensor_tensor(out=a1[:], in0=d1[:], in1=invf[:], op=mybir.AluOpType.mult)

        s1 = acc.tile([BC, J], F32)
        tot1 = acc.tile([BC, 1], F32)
        nc.scalar.activation(
            out=s1[:], in_=a1[:], func=mybir.ActivationFunctionType.Exp,
            scale=1.0 / TEMP, accum_out=tot1[:],
        )
        s2 = acc.tile([BC, J], F32)
        tot2 = acc.tile([BC, 1], F32)
        nc.scalar.activation(
            out=s2[:], in_=d2[:], func=mybir.ActivationFunctionType.Exp,
            scale=1.0 / TEMP, accum_out=tot2[:],
        )

        l1 = acc.tile([BC, 1], F32)
        nc.scalar.activation(out=l1[:], in_=tot1[:], func=mybir.ActivationFunctionType.Ln)
        l2 = acc.tile([BC, 1], F32)
        nc.scalar.activation(out=l2[:], in_=tot2[:], func=mybir.ActivationFunctionType.Ln)

        # c1 partial = ln(tot1) - a1[:,0]/T ; c2 partial = ln(tot2) - d2[:,0]/T
        c1p = acc.tile([BC, 1], F32)
        nc.vector.scalar_tensor_tensor(
            out=c1p[:], in0=a1[:, 0:1], scalar=-1.0 / TEMP, in1=l1[:],
            op0=mybir.AluOpType.mult, op1=mybir.AluOpType.add,
        )
        c2p = acc.tile([BC, 1], F32)
        nc.vector.scalar_tensor_tensor(
            out=c2p[:], in0=d2[:, 0:1], scalar=-1.0 / TEMP, in1=l2[:],
            op0=mybir.AluOpType.mult, op1=mybir.AluOpType.add,
        )

        # reg norms: sum over j of sqrt(n2)
        nu = acc.tile([BC, J], F32)
        su = acc.tile([BC, 1], F32)
        nc.scalar.activation(
            out=nu[:], in_=nu2[:], func=mybir.ActivationFunctionType.Sqrt, accum_out=su[:],
        )
        ne = acc.tile([BC, J], F32)
        se = acc.tile([BC, 1], F32)
        nc.scalar.activation(
            out=ne[:], in_=ne2[:], func=mybir.ActivationFunctionType.Sqrt, accum_out=se[:],
        )

        res = acc.tile([BC, 4], F32)
        nc.vector.tensor_copy(res[:, 0:1], c1p[:])
        nc.vector.tensor_copy(res[:, 1:2], c2p[:])
        nc.vector.tensor_copy(res[:, 2:3], su[:])
        nc.vector.tensor_copy(res[:, 3:4], se[:])
        nc.sync.dma_start(out=out[:, :], in_=res[:])


@functools.cache
def _program():
    return build_program()


def make_in_maps(user_tensor, item_tensor, rand_index, id_embedding, v_feat, W1, b1, W2, b2):
    user_tensor = np.asarray(user_tensor)
    item_tensor = np.asarray(item_tensor)
    rand_index = np.asarray(rand_index)

    mask_flat = np.zeros(FLAT, dtype=np.float32)
    mask_flat[np.asarray(rand_index).astype(np.int64)] = 1.0
    mask = mask_flat.reshape(BATCH, J)

    uidx = user_tensor.astype(np.int32)
    iidx = item_tensor.astype(np.int32)
    vidx = (item_tensor - NUM_USER).astype(np.int32)
    pidx = item_tensor[:, 0:1].astype(np.int32)

    id_embedding = np.ascontiguousarray(np.asarray(id_embedding, dtype=np.float32))
    v_feat = np.ascontiguousarray(np.asarray(v_feat, dtype=np.float32))
    W1 = np.ascontiguousarray(np.asarray(W1, dtype=np.float32))
    W2 = np.ascontiguousarray(np.asarray(W2, dtype=np.float32))
    b1 = np.ascontiguousarray(np.asarray(b1, dtype=np.float32))
    b2 = np.ascontiguousarray(np.asarray(b2, dtype=np.float32))

    in_maps = []
    for c in range(N_CORES):
        rs = slice(c * BC, (c + 1) * BC)
        in_maps.append({
            "uidx": np.ascontiguousarray(uidx[rs]),
            "iidx": np.ascontiguousarray(iidx[rs]),
            "vidx": np.ascontiguousarray(vidx[rs]),
            "pidx": np.ascontiguousarray(pidx[rs]),
            "maskd": np.ascontiguousarray(mask[rs]),
            "id_emb": id_embedding,
            "v_feat": v_feat,
            "w1": W1,
            "w2": W2,
            "b1": b1,
            "b2": b2,
        })
    return in_maps


def combine_partials(parts):
    """parts: list of 8 [128, 4] arrays -> (total, reg_loss) float32 scalars."""
    allp = np.concatenate([np.asarray(p, dtype=np.float64) for p in parts], axis=0)
    c1 = allp[:, 0].sum() / BATCH
    c2 = allp[:, 1].sum() / BATCH
    su = allp[:, 2].sum() / FLAT
    se = allp[:, 3].sum() / FLAT
    total = c1 * LR_LAMBDA + c2 * (1.0 - LR_LAMBDA)
    reg = (su + se) / 2.0
    return np.float32(total), np.float32(reg)


def kernel(user_tensor, item_tensor, rand_index, id_embedding, v_feat, W1, b1, W2, b2):
    from concourse.bass_utils import run_bass_kernel_spmd

    nc = _program()
    in_maps = make_in_maps(user_tensor, item_tensor, rand_index,
                           id_embedding, v_feat, W1, b1, W2, b2)
    res = run_bass_kernel_spmd(nc, in_maps, core_ids=list(range(N_CORES)))
    parts = [res.results[c]["partials"] for c in range(N_CORES)]
    return combine_partials(parts)
